# revision 23
# baseline (speedup 1.0000x reference)
"""Trainium2 Bass kernel for nn_ButterflyProduct (split-compose version).

Math: out_row = T x_row, T = A_0 A_1 ... A_9, A_i = sum_f softmax(logit)[i,f] B_f,
B_f banded with offsets {0, -d_f, +d_f}, d_f = 2^(9-f).

Out = X @ U with U = T^T = A_9^T ... A_0^T.  A^T has the same banded form
with sub/super swapped, so the compose skeleton is reused by swapping the
sb/sp coefficient slots and reversing the logit rows on the host.

Per core c (8 cores):
  1. softmax(logit) via exp -> PE outer-product broadcast -> per-partition
     normalize (no DRAM bounce, no gpsimd).
  2. Build the shear tables for ALL 10 steps in one vectorized pass
     (stride-0 broadcast STT ops over an [i=step, blk] grid), stage each
     step's slim window to its own DRAM buffer, shear-read dense 128x128
     bf16 lhsT blocks with 3-deep buffering.
  3. Compose ONLY U[:, 128c:128c+128]: a [1024, 128] bf16 slab, starting
     from a host-provided identity block einit (nonzero only at tile c),
     through 10 block-banded bf16 matmul steps.  Batch-tile transposes are
     interleaved into the compose to fill PE idle.
  4. One AllGather of the slabs (DRAM collective, flat concat) -> full U.
  5. out[b,:] = x[b,:] @ U: per 128-row batch tile, bf16 matmuls vs U.
"""

import sys

if "/opt/trn_rl_repo" not in sys.path:
    sys.path.insert(0, "/opt/trn_rl_repo")

import numpy as np

SIZE = 1024
MF = 10          # number of butterfly factors
NT = 10          # number of mixing terms
BATCH = 16384
N_CORES = 8
BPC = BATCH // N_CORES   # 2048 rows per core
NB = SIZE // 128         # 8 partition blocks
SLAB = 128               # U columns composed per core
NTILES = BPC // 128      # 16 batch tiles per core
DIAG = [1 << (MF - 1 - f) for f in range(MF)]  # [512,256,128,64,32,16,8,4,2,1]
SMALL_D = [d for d in DIAG if d <= 64]         # [64,32,16,8,4,2,1]
F_OF_D = {DIAG[f]: f for f in range(MF)}
F128, F256, F512 = F_OF_D[128], F_OF_D[256], F_OF_D[512]

# (Delta, Mb) slots for the single-band blocks (d in {256, 512})
SINGLE_BLOCKS = (
    [(2, Mb) for Mb in range(6)]          # slots 0..5   coeff row 0 (S_256)
    + [(-2, Mb) for Mb in range(2, 8)]    # slots 6..11  coeff row 1 (Psh_256)
    + [(4, Mb) for Mb in range(4)]        # slots 12..15 coeff row 2 (S_512)
    + [(-4, Mb) for Mb in range(4, 8)]    # slots 16..19 coeff row 3 (Psh_512)
)
SINGLE_SLOT = {(dl, mb): s for s, (dl, mb) in enumerate(SINGLE_BLOCKS)}
# contiguous Mb runs per coeff row: (crow, mb0, n, slot0)
SINGLE_RUNS = [(0, 0, 6, 0), (1, 2, 6, 6), (2, 0, 4, 12), (3, 4, 4, 16)]

# slim table windows (table col range holding nonzeros; rest stays zero)
WIN = {"C": (64, 193), "P": (1, 129), "M": (128, 256)}

_CACHE = {}


def _build_program():
    import concourse.bacc as bacc
    import concourse.bass as bass
    import concourse.mybir as mybir
    from concourse import tile

    F32 = mybir.dt.float32
    BF16 = mybir.dt.bfloat16
    AX = mybir.AxisListType
    AF = mybir.ActivationFunctionType
    ALU = mybir.AluOpType

    nc = bacc.Bacc("TRN2", target_bir_lowering=False, debug=False, num_devices=N_CORES)

    x_d = nc.dram_tensor("x", [BPC, SIZE], BF16, kind="ExternalInput").ap()
    lg_d = nc.dram_tensor("logit", [NT, MF], F32, kind="ExternalInput").ap()
    dg_d = nc.dram_tensor("dgs", [128, MF, NB], F32, kind="ExternalInput").ap()
    sb_d = nc.dram_tensor("sbc", [128, MF, NB], F32, kind="ExternalInput").ap()
    sp_d = nc.dram_tensor("spsh", [128, MF, NB], F32, kind="ExternalInput").ap()
    id_d = nc.dram_tensor("idstrip", [128, 384], F32, kind="ExternalInput").ap()
    ei_d = nc.dram_tensor("einit", [128, SIZE], F32, kind="ExternalInput").ap()
    out_d = nc.dram_tensor("out", [BPC, SIZE], F32, kind="ExternalOutput").ap()
    # DRAM staging for the shear tables: one buffer per (kind, step)
    stages = {
        (s, st): nc.dram_tensor(f"stg_{s}{st}", [128, NB * 256], BF16).ap()
        for s in "CPM"
        for st in range(NT)
    }
    # collective bounce buffers (flat concat across cores)
    gin_d = nc.dram_tensor("gin", [SIZE * SLAB], BF16).ap()
    gout_d = nc.dram_tensor("gout", [N_CORES * SIZE * SLAB], BF16).ap()

    def shear_src(s, st):
        """AP reading staged tables as dense banded blocks.

        block Mb, row m, col j  <-  stage[m, Mb*256 + 128 + j - m]
        """
        flat = stages[(s, st)].rearrange("a b -> (a b)")
        return bass.AP(
            tensor=flat.tensor,
            offset=128,
            ap=[[NB * 256 - 1, 128], [256, NB], [1, 128]],
        )

    def gather_src(k):
        """U[k][p, 128*c + jj] = gout[c*SIZE*SLAB + (128k + p)*128 + jj]"""
        return bass.AP(
            tensor=gout_d.tensor,
            offset=k * 128 * 128,
            ap=[[128, 128], [SIZE * SLAB, N_CORES], [1, 128]],
        )

    def bcast(ap, pos, n):
        """Insert a stride-0 broadcast dim of size n at free position pos."""
        ap = ap.copy()
        ap.ap = ap.ap[: 1 + pos] + [[0, n]] + ap.ap[1 + pos :]
        return ap

    ncopy = [0]

    def rr_copy(out, in_):
        if ncopy[0] % 2 == 0:
            nc.vector.tensor_copy(out, in_)
        else:
            nc.scalar.copy(out, in_)
        ncopy[0] += 1

    with tile.TileContext(nc) as tc:
        with (
            tc.tile_pool(name="const", bufs=1) as cp,
            tc.tile_pool(name="T", bufs=1) as tp,
        ):
            # ---- load constants ----
            lgf = cp.tile([1, NT * MF], F32, tag="lgf")
            nc.sync.dma_start(lgf[:, :], lg_d.rearrange("a b -> (a b)")[None, :])
            dgs = cp.tile([128, MF, NB], F32, tag="dgs")
            nc.sync.dma_start(dgs[:, :, :], dg_d[:, :, :])
            sbc = cp.tile([128, MF, NB], F32, tag="sbc")
            nc.sync.dma_start(sbc[:, :, :], sb_d[:, :, :])
            spsh = cp.tile([128, MF, NB], F32, tag="spsh")
            nc.sync.dma_start(spsh[:, :, :], sp_d[:, :, :])
            idst = cp.tile([128, 384], F32, tag="idst")
            nc.scalar.dma_start(idst[:, :], id_d[:, :])
            ein = cp.tile([128, SIZE], F32, tag="ein")
            nc.scalar.dma_start(ein[:, :], ei_d[:, :])

            # ---- softmax(logit): exp -> broadcast -> normalize ----
            elg = cp.tile([1, NT * MF], F32, tag="elg")
            nc.scalar.activation(elg[:, :], lgf[:, :], AF.Exp)
            ones1 = cp.tile([1, 128], F32, tag="ones1")
            nc.vector.memset(ones1[:, :], 1.0)
            with tc.tile_pool(name="pps", bufs=1, space="PSUM") as ppsp:
                pps = ppsp.tile([128, NT * MF], F32, tag="pps")
                nc.tensor.matmul(pps[:, :], ones1[:, :], elg[:, :], start=True, stop=True)
                pbce = cp.tile([128, NT, MF], F32, tag="pbce")
                nc.vector.tensor_copy(
                    pbce[:, :, :].rearrange("p a b -> p (a b)"), pps[:, :]
                )
            sm = cp.tile([128, NT, 1], F32, tag="sm")
            nc.vector.reduce_sum(sm[:, :, :], pbce[:, :, :], axis=AX.X)
            rcp = cp.tile([128, NT, 1], F32, tag="rcp")
            nc.vector.reciprocal(rcp[:, :, :], sm[:, :, :])
            pbc = cp.tile([128, NT, MF], F32, tag="pbc")
            for i in range(NT):
                nc.vector.tensor_scalar_mul(pbc[:, i, :], pbce[:, i, :], rcp[:, i, :])

            # ---- vectorized all-steps table build ----
            # prv[:, st, f] = prob used by device step st (= row NT-1-st)
            prv = cp.tile([128, NT, MF], F32, tag="prv")
            for st in range(NT):
                nc.vector.tensor_copy(prv[:, st, :], pbc[:, NT - 1 - st, :])

            tabC = cp.tile([128, NT, NB, 129], BF16, tag="tabC")
            tabP = cp.tile([128, NT, NB, 128], BF16, tag="tabP")
            tabM = cp.tile([128, NT, NB, 128], BF16, tag="tabM")
            nc.vector.memset(tabC[:, :, :, :], 0.0)
            nc.gpsimd.memset(tabP[:, :, :, :], 0.0)
            nc.gpsimd.memset(tabM[:, :, :, :], 0.0)

            def stt_outer(dst, f, coef):
                """dst[m, st, blk] = prv[m, st, f] * coef[m, f, blk]"""
                a = prv[:, :, f : f + 1].copy()
                a.ap = a.ap[:-1]          # [128, NT]
                a = bcast(a, 1, NB)       # [128, NT, NB(0)]
                b = bcast(coef[:, f, :], 0, NT)  # [128, NT(0), NB]
                nc.vector.scalar_tensor_tensor(dst, a, 0.0, b, op0=ALU.add, op1=ALU.mult)

            # D band: tabC[..., 64] = sum_f p * dg  (f32 accum, then copy)
            dacc = cp.tile([128, NT, NB], F32, tag="dacc")
            ttmp = cp.tile([128, NT, NB], F32, tag="ttmp")
            stt_outer(dacc[:, :, :], 0, dgs)
            for f in range(1, MF):
                stt_outer(ttmp[:, :, :], f, dgs)
                nc.vector.scalar_tensor_tensor(
                    dacc[:, :, :], ttmp[:, :, :], 0.0, dacc[:, :, :],
                    op0=ALU.add, op1=ALU.add,
                )
            nc.vector.tensor_copy(tabC[:, :, :, 64], dacc[:, :, :])

            # banded columns (window positions: C 64+-d, P d-1, M 128-d)
            for d in SMALL_D:
                f = F_OF_D[d]
                stt_outer(tabC[:, :, :, 64 + d], f, sbc)
                stt_outer(tabC[:, :, :, 64 - d], f, spsh)
                stt_outer(tabP[:, :, :, d - 1], f, sbc)
                stt_outer(tabM[:, :, :, 128 - d], f, spsh)
            stt_outer(tabP[:, :, :, 127], F128, sbc)
            stt_outer(tabM[:, :, :, 0], F128, spsh)

            # scaled wide-band coefficient columns for all steps
            s4a = cp.tile([128, 4, NT, NB], F32, tag="s4a")
            for crow, f, coef in (
                (0, F256, sbc), (1, F256, spsh), (2, F512, sbc), (3, F512, spsh)
            ):
                a = prv[:, :, f : f + 1].copy()
                a.ap = a.ap[:-1]
                a = bcast(a, 1, NB)
                b = bcast(coef[:, f, :], 0, NT)
                nc.vector.scalar_tensor_tensor(
                    s4a[:, crow, :, :], a, 0.0, b, op0=ALU.add, op1=ALU.mult
                )

            # ---- zero-margined staging scratch (full-width writes) ----
            tabs_all = {"C": tabC, "P": tabP, "M": tabM}
            scr = {
                (s, q2): cp.tile(
                    [128, NB, 256], BF16, tag=f"scr{s}{q2}", name=f"scr{s}{q2}"
                )
                for s in "CPM"
                for q2 in (0, 1, 2)
            }
            for ti, t in enumerate(scr.values()):
                (nc.vector if ti % 2 == 0 else nc.gpsimd).memset(t[:, :, :], 0.0)

            # ---- slab init + shared tiles ----
            Ta = [tp.tile([128, SLAB], BF16, tag=f"Ta{J}", name=f"Ta{J}") for J in range(NB)]
            Tb = [tp.tile([128, SLAB], BF16, tag=f"Tb{J}", name=f"Tb{J}") for J in range(NB)]
            for J in range(NB):
                nc.vector.tensor_copy(Ta[J][:, :], ein[:, 128 * J : 128 * J + 128])
            idb = cp.tile([128, 128], BF16, tag="idb")
            nc.vector.tensor_copy(idb[:, :], idst[:, 127:255])

            U = [tp.tile([128, SIZE], BF16, tag=f"U{K}", name=f"U{K}") for K in range(NB)]
            XT = [
                tp.tile([128, SIZE], BF16, tag=f"XT{t}", name=f"XT{t}")
                for t in range(NTILES)
            ]

            # ---- compose + interleaved x transposes ----
            with (
                tc.tile_pool(name="lhs", bufs=1) as lp,
                tc.tile_pool(name="xin", bufs=3) as xin,
                tc.tile_pool(name="cps", bufs=4, space="PSUM") as cps,
                tc.tile_pool(name="xps", bufs=4, space="PSUM") as xps,
            ):
                lhs = {
                    (s, q): lp.tile([128, NB, 128], BF16, tag=f"lhs{s}{q}", name=f"lhs{s}{q}")
                    for s in "CPM"
                    for q in (0, 1, 2)
                }
                lhsS = {
                    q: lp.tile([128, 20, 128], BF16, tag=f"lhsS{q}", name=f"lhsS{q}")
                    for q in (0, 1, 2)
                }

                def do_xtile(t):
                    xi = xin.tile([128, SIZE], BF16, tag="xi")
                    nc.gpsimd.dma_start(xi[:, :], x_d[128 * t : 128 * t + 128, :])
                    for k in range(NB):
                        tpx = xps.tile([128, 128], BF16, tag="tpx")
                        nc.tensor.transpose(
                            tpx[:, :], xi[:, 128 * k : 128 * k + 128], idb[:, :]
                        )
                        rr_copy(XT[t][:, 128 * k : 128 * k + 128], tpx[:, :])

                def stage_step(st):
                    q = st % 3
                    q2 = st % 3
                    for si, s in enumerate("CPM"):
                        lo, hi = WIN[s]
                        if (st + si) % 2 == 0:
                            nc.vector.tensor_copy(
                                scr[(s, q2)][:, :, lo:hi], tabs_all[s][:, st, :, : hi - lo]
                            )
                        else:
                            nc.scalar.copy(
                                scr[(s, q2)][:, :, lo:hi], tabs_all[s][:, st, :, : hi - lo]
                            )
                        nc.scalar.dma_start(
                            stages[(s, st)][:, :],
                            scr[(s, q2)][:, :, :].rearrange("a b c -> a (b c)"),
                        )
                        nc.sync.dma_start(lhs[(s, q)][:, :, :], shear_src(s, st))

                cur, nxt = Ta, Tb
                xtile_next = 0
                stage_step(0)
                stage_step(1)
                for st in range(NT):
                    if st + 2 < NT:
                        stage_step(st + 2)
                    q = st % 3
                    # wide-band single blocks: 4 vectorized STT builds
                    for crow, mb0, n, slot0 in SINGLE_RUNS:
                        a = bcast(idst[:, 127:255], 0, n)     # [128, n(0), 128]
                        b = s4a[:, crow, st, mb0 : mb0 + n].copy()
                        b.ap = b.ap + [[0, 128]]              # [128, n, 128(0)]
                        nc.vector.scalar_tensor_tensor(
                            lhsS[q][:, slot0 : slot0 + n, :], a, 0.0, b,
                            op0=ALU.add, op1=ALU.mult,
                        )

                    for Jb in range(NB):
                        mms = [(lhs[("C", q)][:, Jb, :], Jb)]
                        if Jb >= 1:
                            mms.append((lhs[("P", q)][:, Jb - 1, :], Jb - 1))
                        if Jb <= 6:
                            mms.append((lhs[("M", q)][:, Jb + 1, :], Jb + 1))
                        for dl in (2, -2, 4, -4):
                            Mb = Jb - dl
                            if 0 <= Mb < NB:
                                mms.append((lhsS[q][:, SINGLE_SLOT[(dl, Mb)], :], Mb))
                        ps = cps.tile([128, SLAB], F32, tag="cacc")
                        for idx, (lh, Mb) in enumerate(mms):
                            nc.tensor.matmul(
                                ps[:, :], lh, cur[Mb][:, :],
                                start=(idx == 0), stop=(idx == len(mms) - 1),
                            )
                        rr_copy(nxt[Jb][:, :], ps[:, :])
                    cur, nxt = nxt, cur

                    # fill PE idle with batch-tile transposes (~2 tiles/step)
                    if xtile_next < NTILES - 6:
                        do_xtile(xtile_next)
                        xtile_next += 1

                # ---- AllGather the slabs -> full U ----
                gin2 = gin_d.rearrange("(a b) -> a b", a=SIZE, b=SLAB)
                for k in range(NB):
                    nc.sync.dma_start(gin2[128 * k : 128 * k + 128, :], cur[k][:, :])
                nc.gpsimd.collective_compute(
                    "AllGather",
                    mybir.AluOpType.bypass,
                    replica_groups=[list(range(N_CORES))],
                    ins=[gin_d.opt()],
                    outs=[gout_d.opt()],
                )
                for k in range(NB):
                    nc.sync.dma_start(U[k][:, :], gather_src(k))

                # remaining transposes run during the gather
                while xtile_next < NTILES:
                    do_xtile(xtile_next)
                    xtile_next += 1

            # ---- batch apply ----
            with (
                tc.tile_pool(name="op", bufs=2) as op,
                tc.tile_pool(name="ops", bufs=4, space="PSUM") as ops,
            ):
                for t in range(NTILES):
                    ob = op.tile([128, SIZE], F32, tag="ob")
                    for h in range(2):
                        ps = ops.tile([128, 512], F32, tag="oacc")
                        for k in range(NB):
                            nc.tensor.matmul(
                                ps[:, :], XT[t][:, 128 * k : 128 * k + 128],
                                U[k][:, 512 * h : 512 * h + 512],
                                start=(k == 0), stop=(k == NB - 1),
                            )
                        rr_copy(ob[:, 512 * h : 512 * h + 512], ps[:, :])
                    nc.scalar.dma_start(out_d[128 * t : 128 * t + 128, :], ob[:, :])

    nc.compile()
    return nc


def _get_program():
    if "nc" not in _CACHE:
        _CACHE["nc"] = _build_program()
    return _CACHE["nc"]


LAST_RESULTS = {}


def kernel(input, diags, subdiags, superdiags, logit, _trace=False):
    import ml_dtypes

    from concourse.bass_utils import run_bass_kernel_spmd

    x = np.ascontiguousarray(
        np.asarray(input, dtype=np.float32).astype(ml_dtypes.bfloat16)
    )
    dg = np.asarray(diags, dtype=np.float32)
    sb = np.asarray(subdiags, dtype=np.float32)
    sp = np.asarray(superdiags, dtype=np.float32)
    lg = np.ascontiguousarray(np.asarray(logit, dtype=np.float32))

    # host staging (pure layout).  The device composes
    #   P = A'_0 A'_1 ... A'_9   (first applied: A'_9)
    # where A'_g is built from prob row g.  We want P = U = T^T, so feed
    #   logit' = logit reversed, sb' <- sp (cleaned), sp' <- sb (shifted).
    sb_for_dev = np.zeros_like(sp)
    sp_for_dev = np.zeros_like(sb)
    for f in range(MF):
        d = DIAG[f]
        sb_for_dev[f, : SIZE - d] = sp[f, : SIZE - d]
        sp_for_dev[f, d:] = sb[f, : SIZE - d]
    lg_dev = np.ascontiguousarray(lg[::-1])

    def pm(v):  # (MF, SIZE) -> (128, MF, NB) with [m, f, blk] = v[f, 128*blk + m]
        return np.ascontiguousarray(v.reshape(MF, NB, 128).transpose(2, 0, 1))

    dgs = pm(dg)
    sbc = pm(sb_for_dev)
    spsh = pm(sp_for_dev)
    idstrip = np.zeros((128, 384), dtype=np.float32)
    for m in range(128):
        idstrip[m, m + 127] = 1.0

    nc = _get_program()
    in_maps = []
    for c in range(N_CORES):
        einit = np.zeros((128, SIZE), dtype=np.float32)
        einit[:, 128 * c : 128 * c + 128] = np.eye(128, dtype=np.float32)
        in_maps.append(
            {
                "x": x[BPC * c : BPC * (c + 1)],
                "logit": lg_dev,
                "dgs": dgs,
                "sbc": sbc,
                "spsh": spsh,
                "idstrip": idstrip,
                "einit": einit,
            }
        )
    res = run_bass_kernel_spmd(nc, in_maps, core_ids=list(range(N_CORES)), trace=_trace)
    LAST_RESULTS["res"] = res
    out = np.concatenate([res.results[c]["out"] for c in range(N_CORES)], axis=0)
    return out


# revision 24
# speedup vs baseline: 1.0355x; 1.0355x over previous
"""Trainium2 Bass kernel for nn_ButterflyProduct (split-compose version).

Math: out_row = T x_row, T = A_0 A_1 ... A_9, A_i = sum_f softmax(logit)[i,f] B_f,
B_f banded with offsets {0, -d_f, +d_f}, d_f = 2^(9-f).

Out = X @ U with U = T^T = A_9^T ... A_0^T.  A^T has the same banded form
with sub/super swapped, so the compose skeleton is reused by swapping the
sb/sp coefficient slots and reversing the logit rows on the host.

Per core c (8 cores):
  1. softmax(logit) via exp -> PE outer-product broadcast -> per-partition
     normalize (no DRAM bounce, no gpsimd).
  2. Build the shear tables for ALL 10 steps in one vectorized pass
     (stride-0 broadcast STT ops over an [i=step, blk] grid), stage each
     step's slim window to its own DRAM buffer, shear-read dense 128x128
     bf16 lhsT blocks with 3-deep buffering.
  3. Compose ONLY U[:, 128c:128c+128]: a [1024, 128] bf16 slab, starting
     from a host-provided identity block einit (nonzero only at tile c),
     through 10 block-banded bf16 matmul steps.  Batch-tile transposes are
     interleaved into the compose to fill PE idle.
  4. One AllGather of the slabs (DRAM collective, flat concat) -> full U.
  5. out[b,:] = x[b,:] @ U: per 128-row batch tile, bf16 matmuls vs U.
"""

import sys

if "/opt/trn_rl_repo" not in sys.path:
    sys.path.insert(0, "/opt/trn_rl_repo")

import numpy as np

SIZE = 1024
MF = 10          # number of butterfly factors
NT = 10          # number of mixing terms
BATCH = 16384
N_CORES = 8
BPC = BATCH // N_CORES   # 2048 rows per core
NB = SIZE // 128         # 8 partition blocks
SLAB = 128               # U columns composed per core
NTILES = BPC // 128      # 16 batch tiles per core
DIAG = [1 << (MF - 1 - f) for f in range(MF)]  # [512,256,128,64,32,16,8,4,2,1]
SMALL_D = [d for d in DIAG if d <= 64]         # [64,32,16,8,4,2,1]
F_OF_D = {DIAG[f]: f for f in range(MF)}
F128, F256, F512 = F_OF_D[128], F_OF_D[256], F_OF_D[512]

# (Delta, Mb) slots for the single-band blocks (d in {256, 512})
SINGLE_BLOCKS = (
    [(2, Mb) for Mb in range(6)]          # slots 0..5   coeff row 0 (S_256)
    + [(-2, Mb) for Mb in range(2, 8)]    # slots 6..11  coeff row 1 (Psh_256)
    + [(4, Mb) for Mb in range(4)]        # slots 12..15 coeff row 2 (S_512)
    + [(-4, Mb) for Mb in range(4, 8)]    # slots 16..19 coeff row 3 (Psh_512)
)
SINGLE_SLOT = {(dl, mb): s for s, (dl, mb) in enumerate(SINGLE_BLOCKS)}
# contiguous Mb runs per coeff row: (crow, mb0, n, slot0)
SINGLE_RUNS = [(0, 0, 6, 0), (1, 2, 6, 6), (2, 0, 4, 12), (3, 4, 4, 16)]

# slim table windows (table col range holding nonzeros; rest stays zero)
WIN = {"C": (64, 193), "P": (1, 129), "M": (128, 256)}

_CACHE = {}


def _build_program():
    import concourse.bacc as bacc
    import concourse.bass as bass
    import concourse.mybir as mybir
    from concourse import tile

    F32 = mybir.dt.float32
    BF16 = mybir.dt.bfloat16
    AX = mybir.AxisListType
    AF = mybir.ActivationFunctionType
    ALU = mybir.AluOpType

    nc = bacc.Bacc("TRN2", target_bir_lowering=False, debug=False, num_devices=N_CORES)

    x_d = nc.dram_tensor("x", [BPC, SIZE], BF16, kind="ExternalInput").ap()
    lg_d = nc.dram_tensor("logit", [NT, MF], F32, kind="ExternalInput").ap()
    dg_d = nc.dram_tensor("dgs", [128, MF, NB], F32, kind="ExternalInput").ap()
    sb_d = nc.dram_tensor("sbc", [128, MF, NB], F32, kind="ExternalInput").ap()
    sp_d = nc.dram_tensor("spsh", [128, MF, NB], F32, kind="ExternalInput").ap()
    id_d = nc.dram_tensor("idstrip", [128, 384], F32, kind="ExternalInput").ap()
    ei_d = nc.dram_tensor("einit", [128, SIZE], F32, kind="ExternalInput").ap()
    out_d = nc.dram_tensor("out", [BPC, SIZE], F32, kind="ExternalOutput").ap()
    # DRAM staging for the shear tables: one buffer per (kind, step)
    stages = {
        (s, st): nc.dram_tensor(f"stg_{s}{st}", [128, NB * 256], BF16).ap()
        for s in "CPM"
        for st in range(NT)
    }
    # collective bounce buffers (flat concat across cores)
    gin_d = nc.dram_tensor("gin", [SIZE * SLAB], BF16).ap()
    gout_d = nc.dram_tensor("gout", [N_CORES * SIZE * SLAB], BF16).ap()

    def shear_src(s, st):
        """AP reading staged tables as dense banded blocks.

        block Mb, row m, col j  <-  stage[m, Mb*256 + 128 + j - m]
        """
        flat = stages[(s, st)].rearrange("a b -> (a b)")
        return bass.AP(
            tensor=flat.tensor,
            offset=128,
            ap=[[NB * 256 - 1, 128], [256, NB], [1, 128]],
        )

    def gather_src(k):
        """U[k][p, 128*c + jj] = gout[c*SIZE*SLAB + (128k + p)*128 + jj]"""
        return bass.AP(
            tensor=gout_d.tensor,
            offset=k * 128 * 128,
            ap=[[128, 128], [SIZE * SLAB, N_CORES], [1, 128]],
        )

    def bcast(ap, pos, n):
        """Insert a stride-0 broadcast dim of size n at free position pos."""
        ap = ap.copy()
        ap.ap = ap.ap[: 1 + pos] + [[0, n]] + ap.ap[1 + pos :]
        return ap

    ncopy = [0]

    def rr_copy(out, in_):
        if ncopy[0] % 2 == 0:
            nc.vector.tensor_copy(out, in_)
        else:
            nc.scalar.copy(out, in_)
        ncopy[0] += 1

    with tile.TileContext(nc) as tc:
        with (
            tc.tile_pool(name="const", bufs=1) as cp,
            tc.tile_pool(name="T", bufs=1) as tp,
        ):
            # ---- load constants ----
            lgf = cp.tile([1, NT * MF], F32, tag="lgf")
            nc.sync.dma_start(lgf[:, :], lg_d.rearrange("a b -> (a b)")[None, :])
            dgs = cp.tile([128, MF, NB], F32, tag="dgs")
            nc.sync.dma_start(dgs[:, :, :], dg_d[:, :, :])
            sbc = cp.tile([128, MF, NB], F32, tag="sbc")
            nc.sync.dma_start(sbc[:, :, :], sb_d[:, :, :])
            spsh = cp.tile([128, MF, NB], F32, tag="spsh")
            nc.sync.dma_start(spsh[:, :, :], sp_d[:, :, :])
            idst = cp.tile([128, 384], F32, tag="idst")
            nc.scalar.dma_start(idst[:, :], id_d[:, :])
            ein = cp.tile([128, SIZE], F32, tag="ein")
            nc.scalar.dma_start(ein[:, :], ei_d[:, :])

            # ---- softmax(logit): exp -> broadcast -> normalize ----
            elg = cp.tile([1, NT * MF], F32, tag="elg")
            nc.scalar.activation(elg[:, :], lgf[:, :], AF.Exp)
            ones1 = cp.tile([1, 128], F32, tag="ones1")
            nc.vector.memset(ones1[:, :], 1.0)
            with tc.tile_pool(name="pps", bufs=1, space="PSUM") as ppsp:
                pps = ppsp.tile([128, NT * MF], F32, tag="pps")
                nc.tensor.matmul(pps[:, :], ones1[:, :], elg[:, :], start=True, stop=True)
                pbce = cp.tile([128, NT, MF], F32, tag="pbce")
                nc.vector.tensor_copy(
                    pbce[:, :, :].rearrange("p a b -> p (a b)"), pps[:, :]
                )
            sm = cp.tile([128, NT, 1], F32, tag="sm")
            nc.vector.reduce_sum(sm[:, :, :], pbce[:, :, :], axis=AX.X)
            rcp = cp.tile([128, NT, 1], F32, tag="rcp")
            nc.vector.reciprocal(rcp[:, :, :], sm[:, :, :])
            pbc = cp.tile([128, NT, MF], F32, tag="pbc")
            for i in range(NT):
                nc.vector.tensor_scalar_mul(pbc[:, i, :], pbce[:, i, :], rcp[:, i, :])

            # ---- vectorized all-steps table build ----
            # prv[:, st, f] = prob used by device step st (= row NT-1-st)
            prv = cp.tile([128, NT, MF], F32, tag="prv")
            for st in range(NT):
                nc.vector.tensor_copy(prv[:, st, :], pbc[:, NT - 1 - st, :])

            tabC = cp.tile([128, NT, NB, 129], BF16, tag="tabC")
            tabP = cp.tile([128, NT, NB, 128], BF16, tag="tabP")
            tabM = cp.tile([128, NT, NB, 128], BF16, tag="tabM")
            nc.vector.memset(tabC[:, :, :, :], 0.0)
            nc.gpsimd.memset(tabP[:, :, :, :], 0.0)
            nc.gpsimd.memset(tabM[:, :, :, :], 0.0)

            def stt_outer(dst, f, coef):
                """dst[m, st, blk] = prv[m, st, f] * coef[m, f, blk]"""
                a = prv[:, :, f : f + 1].copy()
                a.ap = a.ap[:-1]          # [128, NT]
                a = bcast(a, 1, NB)       # [128, NT, NB(0)]
                b = bcast(coef[:, f, :], 0, NT)  # [128, NT(0), NB]
                nc.vector.scalar_tensor_tensor(dst, a, 0.0, b, op0=ALU.add, op1=ALU.mult)

            # D band: tabC[..., 64] = sum_f p * dg  (f32 accum, then copy)
            dacc = cp.tile([128, NT, NB], F32, tag="dacc")
            ttmp = cp.tile([128, NT, NB], F32, tag="ttmp")
            stt_outer(dacc[:, :, :], 0, dgs)
            for f in range(1, MF):
                stt_outer(ttmp[:, :, :], f, dgs)
                nc.vector.scalar_tensor_tensor(
                    dacc[:, :, :], ttmp[:, :, :], 0.0, dacc[:, :, :],
                    op0=ALU.add, op1=ALU.add,
                )
            nc.vector.tensor_copy(tabC[:, :, :, 64], dacc[:, :, :])

            # banded columns (window positions: C 64+-d, P d-1, M 128-d)
            for d in SMALL_D:
                f = F_OF_D[d]
                stt_outer(tabC[:, :, :, 64 + d], f, sbc)
                stt_outer(tabC[:, :, :, 64 - d], f, spsh)
                stt_outer(tabP[:, :, :, d - 1], f, sbc)
                stt_outer(tabM[:, :, :, 128 - d], f, spsh)
            stt_outer(tabP[:, :, :, 127], F128, sbc)
            stt_outer(tabM[:, :, :, 0], F128, spsh)

            # scaled wide-band coefficient columns for all steps
            s4a = cp.tile([128, 4, NT, NB], F32, tag="s4a")
            for crow, f, coef in (
                (0, F256, sbc), (1, F256, spsh), (2, F512, sbc), (3, F512, spsh)
            ):
                a = prv[:, :, f : f + 1].copy()
                a.ap = a.ap[:-1]
                a = bcast(a, 1, NB)
                b = bcast(coef[:, f, :], 0, NT)
                nc.vector.scalar_tensor_tensor(
                    s4a[:, crow, :, :], a, 0.0, b, op0=ALU.add, op1=ALU.mult
                )

            # ---- zero-margined staging scratch (full-width writes) ----
            tabs_all = {"C": tabC, "P": tabP, "M": tabM}
            scr = {
                (s, q2): cp.tile(
                    [128, NB, 256], BF16, tag=f"scr{s}{q2}", name=f"scr{s}{q2}"
                )
                for s in "CPM"
                for q2 in (0, 1)
            }
            for ti, t in enumerate(scr.values()):
                (nc.vector if ti % 2 == 0 else nc.gpsimd).memset(t[:, :, :], 0.0)

            # ---- slab init + shared tiles ----
            Ta = [tp.tile([128, SLAB], BF16, tag=f"Ta{J}", name=f"Ta{J}") for J in range(NB)]
            Tb = [tp.tile([128, SLAB], BF16, tag=f"Tb{J}", name=f"Tb{J}") for J in range(NB)]
            for J in range(NB):
                nc.vector.tensor_copy(Ta[J][:, :], ein[:, 128 * J : 128 * J + 128])
            idb = cp.tile([128, 128], BF16, tag="idb")
            nc.vector.tensor_copy(idb[:, :], idst[:, 127:255])

            U = [tp.tile([128, SIZE], BF16, tag=f"U{K}", name=f"U{K}") for K in range(NB)]
            XT = [
                tp.tile([128, SIZE], BF16, tag=f"XT{t}", name=f"XT{t}")
                for t in range(NTILES)
            ]

            # ---- compose + interleaved x transposes ----
            with (
                tc.tile_pool(name="lhs", bufs=1) as lp,
                tc.tile_pool(name="xin", bufs=3) as xin,
                tc.tile_pool(name="cps", bufs=4, space="PSUM") as cps,
                tc.tile_pool(name="xps", bufs=4, space="PSUM") as xps,
            ):
                lhs = {
                    (s, q): lp.tile([128, NB, 128], BF16, tag=f"lhs{s}{q}", name=f"lhs{s}{q}")
                    for s in "CPM"
                    for q in (0, 1, 2)
                }
                lhsS = {
                    q: lp.tile([128, 20, 128], BF16, tag=f"lhsS{q}", name=f"lhsS{q}")
                    for q in (0, 1, 2)
                }

                def do_xtile(t):
                    xi = xin.tile([128, SIZE], BF16, tag="xi")
                    nc.gpsimd.dma_start(xi[:, :], x_d[128 * t : 128 * t + 128, :])
                    for k in range(NB):
                        tpx = xps.tile([128, 128], BF16, tag="tpx")
                        nc.tensor.transpose(
                            tpx[:, :], xi[:, 128 * k : 128 * k + 128], idb[:, :]
                        )
                        rr_copy(XT[t][:, 128 * k : 128 * k + 128], tpx[:, :])

                def stage_step(st):
                    q = st % 3
                    q2 = st % 2
                    for si, s in enumerate("CPM"):
                        lo, hi = WIN[s]
                        if (st + si) % 2 == 0:
                            nc.vector.tensor_copy(
                                scr[(s, q2)][:, :, lo:hi], tabs_all[s][:, st, :, : hi - lo]
                            )
                        else:
                            nc.scalar.copy(
                                scr[(s, q2)][:, :, lo:hi], tabs_all[s][:, st, :, : hi - lo]
                            )
                        nc.scalar.dma_start(
                            stages[(s, st)][:, :],
                            scr[(s, q2)][:, :, :].rearrange("a b c -> a (b c)"),
                        )
                        nc.sync.dma_start(lhs[(s, q)][:, :, :], shear_src(s, st))

                cur, nxt = Ta, Tb
                xtile_next = 0
                stage_step(0)
                stage_step(1)
                for st in range(NT):
                    if st + 2 < NT:
                        stage_step(st + 2)
                    q = st % 3
                    # wide-band single blocks: 4 vectorized STT builds
                    for crow, mb0, n, slot0 in SINGLE_RUNS:
                        a = bcast(idst[:, 127:255], 0, n)     # [128, n(0), 128]
                        b = s4a[:, crow, st, mb0 : mb0 + n].copy()
                        b.ap = b.ap + [[0, 128]]              # [128, n, 128(0)]
                        nc.vector.scalar_tensor_tensor(
                            lhsS[q][:, slot0 : slot0 + n, :], a, 0.0, b,
                            op0=ALU.add, op1=ALU.mult,
                        )

                    for Jb in range(NB):
                        mms = [(lhs[("C", q)][:, Jb, :], Jb)]
                        if Jb >= 1:
                            mms.append((lhs[("P", q)][:, Jb - 1, :], Jb - 1))
                        if Jb <= 6:
                            mms.append((lhs[("M", q)][:, Jb + 1, :], Jb + 1))
                        for dl in (2, -2, 4, -4):
                            Mb = Jb - dl
                            if 0 <= Mb < NB:
                                mms.append((lhsS[q][:, SINGLE_SLOT[(dl, Mb)], :], Mb))
                        ps = cps.tile([128, SLAB], F32, tag="cacc")
                        for idx, (lh, Mb) in enumerate(mms):
                            nc.tensor.matmul(
                                ps[:, :], lh, cur[Mb][:, :],
                                start=(idx == 0), stop=(idx == len(mms) - 1),
                            )
                        rr_copy(nxt[Jb][:, :], ps[:, :])
                    cur, nxt = nxt, cur

                    # fill PE idle with batch-tile transposes (~2 tiles/step)
                    if xtile_next < NTILES - 6:
                        do_xtile(xtile_next)
                        xtile_next += 1

                # ---- AllGather the slabs -> full U ----
                gin2 = gin_d.rearrange("(a b) -> a b", a=SIZE, b=SLAB)
                for k in range(NB):
                    nc.sync.dma_start(gin2[128 * k : 128 * k + 128, :], cur[k][:, :])
                nc.gpsimd.collective_compute(
                    "AllGather",
                    mybir.AluOpType.bypass,
                    replica_groups=[list(range(N_CORES))],
                    ins=[gin_d.opt()],
                    outs=[gout_d.opt()],
                )
                for k in range(NB):
                    nc.sync.dma_start(U[k][:, :], gather_src(k))

                # remaining transposes run during the gather
                while xtile_next < NTILES:
                    do_xtile(xtile_next)
                    xtile_next += 1

            # ---- batch apply ----
            with (
                tc.tile_pool(name="op", bufs=2) as op,
                tc.tile_pool(name="ops", bufs=4, space="PSUM") as ops,
            ):
                for t in range(NTILES):
                    ob = op.tile([128, SIZE], F32, tag="ob")
                    for h in range(2):
                        ps = ops.tile([128, 512], F32, tag="oacc")
                        for k in range(NB):
                            nc.tensor.matmul(
                                ps[:, :], XT[t][:, 128 * k : 128 * k + 128],
                                U[k][:, 512 * h : 512 * h + 512],
                                start=(k == 0), stop=(k == NB - 1),
                            )
                        rr_copy(ob[:, 512 * h : 512 * h + 512], ps[:, :])
                    nc.scalar.dma_start(out_d[128 * t : 128 * t + 128, :], ob[:, :])

    nc.compile()
    return nc


def _get_program():
    if "nc" not in _CACHE:
        _CACHE["nc"] = _build_program()
    return _CACHE["nc"]


LAST_RESULTS = {}


def kernel(input, diags, subdiags, superdiags, logit, _trace=False):
    import ml_dtypes

    from concourse.bass_utils import run_bass_kernel_spmd

    x = np.ascontiguousarray(
        np.asarray(input, dtype=np.float32).astype(ml_dtypes.bfloat16)
    )
    dg = np.asarray(diags, dtype=np.float32)
    sb = np.asarray(subdiags, dtype=np.float32)
    sp = np.asarray(superdiags, dtype=np.float32)
    lg = np.ascontiguousarray(np.asarray(logit, dtype=np.float32))

    # host staging (pure layout).  The device composes
    #   P = A'_0 A'_1 ... A'_9   (first applied: A'_9)
    # where A'_g is built from prob row g.  We want P = U = T^T, so feed
    #   logit' = logit reversed, sb' <- sp (cleaned), sp' <- sb (shifted).
    sb_for_dev = np.zeros_like(sp)
    sp_for_dev = np.zeros_like(sb)
    for f in range(MF):
        d = DIAG[f]
        sb_for_dev[f, : SIZE - d] = sp[f, : SIZE - d]
        sp_for_dev[f, d:] = sb[f, : SIZE - d]
    lg_dev = np.ascontiguousarray(lg[::-1])

    def pm(v):  # (MF, SIZE) -> (128, MF, NB) with [m, f, blk] = v[f, 128*blk + m]
        return np.ascontiguousarray(v.reshape(MF, NB, 128).transpose(2, 0, 1))

    dgs = pm(dg)
    sbc = pm(sb_for_dev)
    spsh = pm(sp_for_dev)
    idstrip = np.zeros((128, 384), dtype=np.float32)
    for m in range(128):
        idstrip[m, m + 127] = 1.0

    nc = _get_program()
    in_maps = []
    for c in range(N_CORES):
        einit = np.zeros((128, SIZE), dtype=np.float32)
        einit[:, 128 * c : 128 * c + 128] = np.eye(128, dtype=np.float32)
        in_maps.append(
            {
                "x": x[BPC * c : BPC * (c + 1)],
                "logit": lg_dev,
                "dgs": dgs,
                "sbc": sbc,
                "spsh": spsh,
                "idstrip": idstrip,
                "einit": einit,
            }
        )
    res = run_bass_kernel_spmd(nc, in_maps, core_ids=list(range(N_CORES)), trace=_trace)
    LAST_RESULTS["res"] = res
    out = np.concatenate([res.results[c]["out"] for c in range(N_CORES)], axis=0)
    return out


# revision 27
# speedup vs baseline: 1.0544x; 1.0183x over previous
"""Trainium2 Bass kernel for nn_ButterflyProduct (split-compose version).

Math: out_row = T x_row, T = A_0 A_1 ... A_9, A_i = sum_f softmax(logit)[i,f] B_f,
B_f banded with offsets {0, -d_f, +d_f}, d_f = 2^(9-f).

Out = X @ U with U = T^T = A_9^T ... A_0^T.  A^T has the same banded form
with sub/super swapped, so the compose skeleton is reused by swapping the
sb/sp coefficient slots and reversing the logit rows on the host.

Per core c (8 cores):
  1. softmax(logit) via exp -> PE outer-product broadcast -> per-partition
     normalize (no DRAM bounce, no gpsimd).
  2. Build the shear tables for ALL 10 steps in one vectorized pass
     (stride-0 broadcast STT ops over an [i=step, blk] grid), stage each
     step's slim window to its own DRAM buffer, shear-read dense 128x128
     bf16 lhsT blocks with 3-deep buffering.
  3. Compose ONLY U[:, 128c:128c+128]: a [1024, 128] bf16 slab, starting
     from a host-provided identity block einit (nonzero only at tile c),
     through 10 block-banded bf16 matmul steps.  Batch-tile transposes are
     interleaved into the compose to fill PE idle.
  4. One AllGather of the slabs (DRAM collective, flat concat) -> full U.
  5. out[b,:] = x[b,:] @ U: per 128-row batch tile, bf16 matmuls vs U.
"""

import sys

if "/opt/trn_rl_repo" not in sys.path:
    sys.path.insert(0, "/opt/trn_rl_repo")

import numpy as np

SIZE = 1024
MF = 10          # number of butterfly factors
NT = 10          # number of mixing terms
BATCH = 16384
N_CORES = 8
BPC = BATCH // N_CORES   # 2048 rows per core
NB = SIZE // 128         # 8 partition blocks
SLAB = 128               # U columns composed per core
NTILES = BPC // 128      # 16 batch tiles per core
DIAG = [1 << (MF - 1 - f) for f in range(MF)]  # [512,256,128,64,32,16,8,4,2,1]
SMALL_D = [d for d in DIAG if d <= 64]         # [64,32,16,8,4,2,1]
F_OF_D = {DIAG[f]: f for f in range(MF)}
F128, F256, F512 = F_OF_D[128], F_OF_D[256], F_OF_D[512]

# (Delta, Mb) slots for the single-band blocks (d in {256, 512})
SINGLE_BLOCKS = (
    [(2, Mb) for Mb in range(6)]          # slots 0..5   coeff row 0 (S_256)
    + [(-2, Mb) for Mb in range(2, 8)]    # slots 6..11  coeff row 1 (Psh_256)
    + [(4, Mb) for Mb in range(4)]        # slots 12..15 coeff row 2 (S_512)
    + [(-4, Mb) for Mb in range(4, 8)]    # slots 16..19 coeff row 3 (Psh_512)
)
SINGLE_SLOT = {(dl, mb): s for s, (dl, mb) in enumerate(SINGLE_BLOCKS)}
# contiguous Mb runs per coeff row: (crow, mb0, n, slot0)
SINGLE_RUNS = [(0, 0, 6, 0), (1, 2, 6, 6), (2, 0, 4, 12), (3, 4, 4, 16)]

# slim table windows (table col range holding nonzeros; rest stays zero)
WIN = {"C": (64, 193), "P": (1, 129), "M": (128, 256)}

_CACHE = {}


def _build_program():
    import concourse.bacc as bacc
    import concourse.bass as bass
    import concourse.mybir as mybir
    from concourse import tile

    F32 = mybir.dt.float32
    BF16 = mybir.dt.bfloat16
    AX = mybir.AxisListType
    AF = mybir.ActivationFunctionType
    ALU = mybir.AluOpType

    nc = bacc.Bacc("TRN2", target_bir_lowering=False, debug=False, num_devices=N_CORES)

    x_d = nc.dram_tensor("x", [BPC, SIZE], BF16, kind="ExternalInput").ap()
    lg_d = nc.dram_tensor("logit", [NT, MF], F32, kind="ExternalInput").ap()
    dg_d = nc.dram_tensor("dgs", [128, MF, NB], F32, kind="ExternalInput").ap()
    sb_d = nc.dram_tensor("sbc", [128, MF, NB], F32, kind="ExternalInput").ap()
    sp_d = nc.dram_tensor("spsh", [128, MF, NB], F32, kind="ExternalInput").ap()
    id_d = nc.dram_tensor("idstrip", [128, 384], F32, kind="ExternalInput").ap()
    ei_d = nc.dram_tensor("einit", [128, SIZE], F32, kind="ExternalInput").ap()
    out_d = nc.dram_tensor("out", [BPC, SIZE], F32, kind="ExternalOutput").ap()
    # DRAM staging for the shear tables: one buffer per (kind, step)
    stages = {
        (s, st): nc.dram_tensor(f"stg_{s}{st}", [128, NB * 256], BF16).ap()
        for s in "CPM"
        for st in range(NT)
    }
    # collective bounce buffers (flat concat across cores)
    gin_d = nc.dram_tensor("gin", [SIZE * SLAB], BF16).ap()
    gout_d = nc.dram_tensor("gout", [N_CORES * SIZE * SLAB], BF16).ap()

    def shear_src(s, st):
        """AP reading staged tables as dense banded blocks.

        block Mb, row m, col j  <-  stage[m, Mb*256 + 128 + j - m]
        """
        flat = stages[(s, st)].rearrange("a b -> (a b)")
        return bass.AP(
            tensor=flat.tensor,
            offset=128,
            ap=[[NB * 256 - 1, 128], [256, NB], [1, 128]],
        )

    def gather_src(k):
        """U[k][p, 128*c + jj] = gout[c*SIZE*SLAB + (128k + p)*128 + jj]"""
        return bass.AP(
            tensor=gout_d.tensor,
            offset=k * 128 * 128,
            ap=[[128, 128], [SIZE * SLAB, N_CORES], [1, 128]],
        )

    def bcast(ap, pos, n):
        """Insert a stride-0 broadcast dim of size n at free position pos."""
        ap = ap.copy()
        ap.ap = ap.ap[: 1 + pos] + [[0, n]] + ap.ap[1 + pos :]
        return ap

    ncopy = [0]

    def rr_copy(out, in_):
        if ncopy[0] % 2 == 0:
            nc.vector.tensor_copy(out, in_)
        else:
            nc.scalar.copy(out, in_)
        ncopy[0] += 1

    with tile.TileContext(nc) as tc:
        with (
            tc.tile_pool(name="const", bufs=1) as cp,
            tc.tile_pool(name="T", bufs=1) as tp,
        ):
            # ---- load constants ----
            lgf = cp.tile([1, NT * MF], F32, tag="lgf")
            nc.sync.dma_start(lgf[:, :], lg_d.rearrange("a b -> (a b)")[None, :])
            dgs = cp.tile([128, MF, NB], F32, tag="dgs")
            nc.sync.dma_start(dgs[:, :, :], dg_d[:, :, :])
            sbc = cp.tile([128, MF, NB], F32, tag="sbc")
            nc.sync.dma_start(sbc[:, :, :], sb_d[:, :, :])
            spsh = cp.tile([128, MF, NB], F32, tag="spsh")
            nc.sync.dma_start(spsh[:, :, :], sp_d[:, :, :])
            idst = cp.tile([128, 384], F32, tag="idst")
            nc.scalar.dma_start(idst[:, :], id_d[:, :])
            ein = cp.tile([128, SIZE], F32, tag="ein")
            nc.scalar.dma_start(ein[:, :], ei_d[:, :])

            # ---- softmax(logit): exp -> broadcast -> normalize ----
            elg = cp.tile([1, NT * MF], F32, tag="elg")
            nc.scalar.activation(elg[:, :], lgf[:, :], AF.Exp)
            ones1 = cp.tile([1, 128], F32, tag="ones1")
            nc.vector.memset(ones1[:, :], 1.0)
            with tc.tile_pool(name="pps", bufs=1, space="PSUM") as ppsp:
                pps = ppsp.tile([128, NT * MF], F32, tag="pps")
                nc.tensor.matmul(pps[:, :], ones1[:, :], elg[:, :], start=True, stop=True)
                pbce = cp.tile([128, NT, MF], F32, tag="pbce")
                nc.vector.tensor_copy(
                    pbce[:, :, :].rearrange("p a b -> p (a b)"), pps[:, :]
                )
            sm = cp.tile([128, NT, 1], F32, tag="sm")
            nc.vector.reduce_sum(sm[:, :, :], pbce[:, :, :], axis=AX.X)
            rcp = cp.tile([128, NT, 1], F32, tag="rcp")
            nc.vector.reciprocal(rcp[:, :, :], sm[:, :, :])
            pbc = cp.tile([128, NT, MF], F32, tag="pbc")
            for i in range(NT):
                nc.vector.tensor_scalar_mul(pbc[:, i, :], pbce[:, i, :], rcp[:, i, :])

            # ---- vectorized all-steps table build ----
            # prv[:, st, f] = prob used by device step st (= row NT-1-st)
            prv = cp.tile([128, NT, MF], F32, tag="prv")
            for st in range(NT):
                nc.vector.tensor_copy(prv[:, st, :], pbc[:, NT - 1 - st, :])

            tabC = cp.tile([128, NT, NB, 129], BF16, tag="tabC")
            tabP = cp.tile([128, NT, NB, 128], BF16, tag="tabP")
            tabM = cp.tile([128, NT, NB, 128], BF16, tag="tabM")
            nc.vector.memset(tabC[:, :, :, :], 0.0)
            nc.gpsimd.memset(tabP[:, :, :, :], 0.0)
            nc.gpsimd.memset(tabM[:, :, :, :], 0.0)

            def stt_outer(dst, f, coef):
                """dst[m, st, blk] = prv[m, st, f] * coef[m, f, blk]"""
                a = prv[:, :, f : f + 1].copy()
                a.ap = a.ap[:-1]          # [128, NT]
                a = bcast(a, 1, NB)       # [128, NT, NB(0)]
                b = bcast(coef[:, f, :], 0, NT)  # [128, NT(0), NB]
                nc.vector.scalar_tensor_tensor(dst, a, 0.0, b, op0=ALU.add, op1=ALU.mult)

            # D band: tabC[..., 64] = sum_f p * dg  (f32 accum, then copy)
            dacc = cp.tile([128, NT, NB], F32, tag="dacc")
            ttmp = cp.tile([128, NT, NB], F32, tag="ttmp")
            stt_outer(dacc[:, :, :], 0, dgs)
            for f in range(1, MF):
                stt_outer(ttmp[:, :, :], f, dgs)
                nc.vector.scalar_tensor_tensor(
                    dacc[:, :, :], ttmp[:, :, :], 0.0, dacc[:, :, :],
                    op0=ALU.add, op1=ALU.add,
                )
            nc.vector.tensor_copy(tabC[:, :, :, 64], dacc[:, :, :])

            # banded columns (window positions: C 64+-d, P d-1, M 128-d)
            for d in SMALL_D:
                f = F_OF_D[d]
                stt_outer(tabC[:, :, :, 64 + d], f, sbc)
                stt_outer(tabC[:, :, :, 64 - d], f, spsh)
                stt_outer(tabP[:, :, :, d - 1], f, sbc)
                stt_outer(tabM[:, :, :, 128 - d], f, spsh)
            stt_outer(tabP[:, :, :, 127], F128, sbc)
            stt_outer(tabM[:, :, :, 0], F128, spsh)

            # scaled wide-band coefficient columns for all steps
            s4a = cp.tile([128, 4, NT, NB], F32, tag="s4a")
            for crow, f, coef in (
                (0, F256, sbc), (1, F256, spsh), (2, F512, sbc), (3, F512, spsh)
            ):
                a = prv[:, :, f : f + 1].copy()
                a.ap = a.ap[:-1]
                a = bcast(a, 1, NB)
                b = bcast(coef[:, f, :], 0, NT)
                nc.vector.scalar_tensor_tensor(
                    s4a[:, crow, :, :], a, 0.0, b, op0=ALU.add, op1=ALU.mult
                )

            # ---- zero-margined staging scratch (full-width writes) ----
            tabs_all = {"C": tabC, "P": tabP, "M": tabM}
            scr = {
                (s, q2): cp.tile(
                    [128, NB, 256], BF16, tag=f"scr{s}{q2}", name=f"scr{s}{q2}"
                )
                for s in "CPM"
                for q2 in (0, 1)
            }
            for ti, t in enumerate(scr.values()):
                (nc.vector if ti % 2 == 0 else nc.gpsimd).memset(t[:, :, :], 0.0)

            # ---- slab init + shared tiles ----
            Ta = [tp.tile([128, SLAB], BF16, tag=f"Ta{J}", name=f"Ta{J}") for J in range(NB)]
            Tb = [tp.tile([128, SLAB], BF16, tag=f"Tb{J}", name=f"Tb{J}") for J in range(NB)]
            for J in range(NB):
                nc.vector.tensor_copy(Ta[J][:, :], ein[:, 128 * J : 128 * J + 128])
            idb = cp.tile([128, 128], BF16, tag="idb")
            nc.vector.tensor_copy(idb[:, :], idst[:, 127:255])

            U = [tp.tile([128, SIZE], BF16, tag=f"U{K}", name=f"U{K}") for K in range(NB)]
            XT = [
                tp.tile([128, SIZE], BF16, tag=f"XT{t}", name=f"XT{t}")
                for t in range(NTILES)
            ]

            # ---- compose + interleaved x transposes ----
            with (
                tc.tile_pool(name="lhs", bufs=1) as lp,
                tc.tile_pool(name="xin", bufs=3) as xin,
                tc.tile_pool(name="cps", bufs=4, space="PSUM") as cps,
                tc.tile_pool(name="xps", bufs=4, space="PSUM") as xps,
            ):
                lhs = {
                    (s, q): lp.tile([128, NB, 128], BF16, tag=f"lhs{s}{q}", name=f"lhs{s}{q}")
                    for s in "CPM"
                    for q in (0, 1, 2)
                }
                lhsS = {
                    q: lp.tile([128, 20, 128], BF16, tag=f"lhsS{q}", name=f"lhsS{q}")
                    for q in (0, 1, 2)
                }

                def do_xtile(t):
                    xi = xin.tile([128, SIZE], BF16, tag="xi")
                    nc.gpsimd.dma_start(xi[:, :], x_d[128 * t : 128 * t + 128, :])
                    for k in range(NB):
                        tpx = xps.tile([128, 128], BF16, tag="tpx")
                        nc.tensor.transpose(
                            tpx[:, :], xi[:, 128 * k : 128 * k + 128], idb[:, :]
                        )
                        rr_copy(XT[t][:, 128 * k : 128 * k + 128], tpx[:, :])

                def stage_step(st):
                    q = st % 3
                    q2 = st % 2
                    for si, s in enumerate("CPM"):
                        lo, hi = WIN[s]
                        if (st + si) % 2 == 0:
                            nc.vector.tensor_copy(
                                scr[(s, q2)][:, :, lo:hi], tabs_all[s][:, st, :, : hi - lo]
                            )
                        else:
                            nc.scalar.copy(
                                scr[(s, q2)][:, :, lo:hi], tabs_all[s][:, st, :, : hi - lo]
                            )
                        nc.scalar.dma_start(
                            stages[(s, st)][:, :],
                            scr[(s, q2)][:, :, :].rearrange("a b c -> a (b c)"),
                        )
                        nc.sync.dma_start(lhs[(s, q)][:, :, :], shear_src(s, st))

                cur, nxt = Ta, Tb
                xtile_next = 0
                stage_step(0)
                stage_step(1)
                for st in range(NT):
                    if st + 2 < NT:
                        stage_step(st + 2)
                    q = st % 3
                    # wide-band single blocks: 4 vectorized STT builds
                    for crow, mb0, n, slot0 in SINGLE_RUNS:
                        a = bcast(idst[:, 127:255], 0, n)     # [128, n(0), 128]
                        b = s4a[:, crow, st, mb0 : mb0 + n].copy()
                        b.ap = b.ap + [[0, 128]]              # [128, n, 128(0)]
                        nc.vector.scalar_tensor_tensor(
                            lhsS[q][:, slot0 : slot0 + n, :], a, 0.0, b,
                            op0=ALU.add, op1=ALU.mult,
                        )

                    for Jb in range(NB):
                        mms = [(lhs[("C", q)][:, Jb, :], Jb)]
                        if Jb >= 1:
                            mms.append((lhs[("P", q)][:, Jb - 1, :], Jb - 1))
                        if Jb <= 6:
                            mms.append((lhs[("M", q)][:, Jb + 1, :], Jb + 1))
                        for dl in (2, -2, 4, -4):
                            Mb = Jb - dl
                            if 0 <= Mb < NB:
                                mms.append((lhsS[q][:, SINGLE_SLOT[(dl, Mb)], :], Mb))
                        ps = cps.tile([128, SLAB], F32, tag="cacc")
                        for idx, (lh, Mb) in enumerate(mms):
                            nc.tensor.matmul(
                                ps[:, :], lh, cur[Mb][:, :],
                                start=(idx == 0), stop=(idx == len(mms) - 1),
                            )
                        rr_copy(nxt[Jb][:, :], ps[:, :])
                    cur, nxt = nxt, cur

                    # fill PE idle with batch-tile transposes (~2 tiles/step)
                    if xtile_next < NTILES - 6:
                        do_xtile(xtile_next)
                        xtile_next += 1

                # ---- AllGather the slabs -> full U ----
                gin2 = gin_d.rearrange("(a b) -> a b", a=SIZE, b=SLAB)
                for k in range(NB):
                    nc.sync.dma_start(gin2[128 * k : 128 * k + 128, :], cur[k][:, :])
                nc.gpsimd.collective_compute(
                    "AllGather",
                    mybir.AluOpType.bypass,
                    replica_groups=[list(range(N_CORES))],
                    ins=[gin_d.opt()],
                    outs=[gout_d.opt()],
                )
                for k in range(NB):
                    nc.sync.dma_start(U[k][:, :], gather_src(k))

                # remaining transposes run during the gather
                while xtile_next < NTILES:
                    do_xtile(xtile_next)
                    xtile_next += 1

            # ---- batch apply ----
            with (
                tc.tile_pool(name="op", bufs=2) as op,
                tc.tile_pool(name="ops", bufs=4, space="PSUM") as ops,
            ):
                for t in range(NTILES):
                    ob = op.tile([128, SIZE], F32, tag="ob")
                    for h in range(2):
                        ps = ops.tile([128, 512], F32, tag="oacc")
                        for k in range(NB):
                            nc.tensor.matmul(
                                ps[:, :], XT[t][:, 128 * k : 128 * k + 128],
                                U[k][:, 512 * h : 512 * h + 512],
                                start=(k == 0), stop=(k == NB - 1),
                            )
                        rr_copy(ob[:, 512 * h : 512 * h + 512], ps[:, :])
                    nc.scalar.dma_start(out_d[128 * t : 128 * t + 128, :], ob[:, :])

    nc.compile()
    return nc


def _get_program():
    if "nc" not in _CACHE:
        _CACHE["nc"] = _build_program()
    return _CACHE["nc"]


LAST_RESULTS = {}


def kernel(input, diags, subdiags, superdiags, logit, _trace=False):
    import ml_dtypes

    from concourse.bass_utils import run_bass_kernel_spmd

    x = np.ascontiguousarray(
        np.asarray(input, dtype=np.float32).astype(ml_dtypes.bfloat16)
    )
    dg = np.asarray(diags, dtype=np.float32)
    sb = np.asarray(subdiags, dtype=np.float32)
    sp = np.asarray(superdiags, dtype=np.float32)
    lg = np.ascontiguousarray(np.asarray(logit, dtype=np.float32))

    # host staging (pure layout).  The device composes
    #   P = A'_0 A'_1 ... A'_9   (first applied: A'_9)
    # where A'_g is built from prob row g.  We want P = U = T^T, so feed
    #   logit' = logit reversed, sb' <- sp (cleaned), sp' <- sb (shifted).
    sb_for_dev = np.zeros_like(sp)
    sp_for_dev = np.zeros_like(sb)
    for f in range(MF):
        d = DIAG[f]
        sb_for_dev[f, : SIZE - d] = sp[f, : SIZE - d]
        sp_for_dev[f, d:] = sb[f, : SIZE - d]
    lg_dev = np.ascontiguousarray(lg[::-1])

    def pm(v):  # (MF, SIZE) -> (128, MF, NB) with [m, f, blk] = v[f, 128*blk + m]
        return np.ascontiguousarray(v.reshape(MF, NB, 128).transpose(2, 0, 1))

    dgs = pm(dg)
    sbc = pm(sb_for_dev)
    spsh = pm(sp_for_dev)
    idstrip = np.zeros((128, 384), dtype=np.float32)
    for m in range(128):
        idstrip[m, m + 127] = 1.0

    nc = _get_program()
    in_maps = []
    for c in range(N_CORES):
        einit = np.zeros((128, SIZE), dtype=np.float32)
        einit[:, 128 * c : 128 * c + 128] = np.eye(128, dtype=np.float32)
        in_maps.append(
            {
                "x": x[BPC * c : BPC * (c + 1)],
                "logit": lg_dev,
                "dgs": dgs,
                "sbc": sbc,
                "spsh": spsh,
                "idstrip": idstrip,
                "einit": einit,
            }
        )
    res = run_bass_kernel_spmd(nc, in_maps, core_ids=list(range(N_CORES)), trace=_trace)
    LAST_RESULTS["res"] = res
    out = np.concatenate([res.results[c]["out"] for c in range(N_CORES)], axis=0)
    return out
